# revision 24
# baseline (speedup 1.0000x reference)
"""Trainium2 Bass kernel for the SelfOrg spiking-network step.

Reference computation (per batch b, neuron n):
    z_out_new = BETA * z_out + z
    z_loo[b,j,n] = z_out_new[b, j + (j>=n)]            (leave-one-out gather)
    drive[b,n]  = sum_k x[b,k,n] * w[k,n]  (k < N_IN)
                + sum_j z_loo[b,j,n] * w[N_IN+j, n]
    v_new = ALPHA*v + drive - V_TH*z
    z_new = (v_new - V_TH > 0)

Strategy:
  * Batch-parallel over 8 cores (8 batches each).
  * The x-part is an elementwise-weighted reduction over k. Layout: k on
    SBUF partitions (p = k//16, s = k%16), n in the free dim. The vector
    engine does tmp = x*w in-place; the tensor engine reduces over
    partitions with a per-batch indicator stationary operand
    (lhsT[p, m] = (m==b)), accumulating all batches into one (8,512)
    PSUM tile with b on partitions.
  * The leave-one-out term is algebraically a dense matmul
    z_out_new @ Wf where Wf[m,n] = w[N_IN + m - (m>n), n], diag(Wf)=0.
    Wf is precomputed on the host; the (8,512)x(512,512) matmul runs on
    the tensor engine using 4 PE transposes of z_out_new as lhsT.

  Note on sync waits: TRN2 matmul / tensor-scalar instructions have a
  single sync-wait slot, so the kernel is arranged so each such
  instruction depends on at most one foreign semaphore: small constants
  (ident, ind) are built on the vector engine (sharing the DVE sem with
  zon), and two tiny "absorber" ops pre-wait the w/wf DMA semaphores.
"""

import numpy as np

# model hyperparameters (must match the reference)
N_IN = 2048
NN = 512
BATCH = 64
DT, TAU_M, TAU_X = 0.05, 10.0, 2.0
ALPHA = 1.0 - DT / TAU_M
BETA = 1.0 - DT / TAU_X
V_TH = 2.0

NCORES = 8
BPC = BATCH // NCORES      # batches per core
P = 128                    # SBUF partitions
S = N_IN // P              # 16 k-rows folded per partition
FD = S * NN                # 8192 free elements of one batch tile
CHUNKS = 4                 # DMA / vector-multiply chunks per batch
CFD = FD // CHUNKS         # 2048 free elements per chunk
SPC = S // CHUNKS          # 4 reduce slices per chunk
XBUFS = 10                 # x chunk tiles in flight (DMA ahead of DVE)
TBUFS = 6                  # product chunk tiles in flight (DVE ahead of PE)


def _build_nc():
    import concourse.mybir as mybir
    from concourse import bacc
    from concourse.masks import make_identity
    from concourse.tile import TileContext

    f32 = mybir.dt.float32
    nc = bacc.Bacc("TRN2", name="selforg_step")

    x_h = nc.dram_tensor("x", [BPC, N_IN, NN], f32, kind="ExternalInput")
    v_h = nc.dram_tensor("v", [BPC, NN], f32, kind="ExternalInput")
    z_h = nc.dram_tensor("z", [BPC, NN], f32, kind="ExternalInput")
    zo_h = nc.dram_tensor("z_out", [BPC, NN], f32, kind="ExternalInput")
    w_h = nc.dram_tensor("w", [N_IN, NN], f32, kind="ExternalInput")
    wf_h = nc.dram_tensor("wf", [NN, NN], f32, kind="ExternalInput")
    out_h = nc.dram_tensor("out", [3, BPC, NN], f32, kind="ExternalOutput")

    # partition p <- x[b] bytes [32KB*p, 32KB*(p+1)): k = 16p + s
    x_r = x_h[:, :, :].rearrange("b (p s) n -> b p (s n)", p=P)
    w_r = w_h[:, :].rearrange("(p s) n -> p (s n)", p=P)
    wf_r = wf_h[:, :].rearrange("(t p) n -> p t n", p=P)

    with TileContext(nc) as tc:
        with (
            tc.tile_pool(name="const", bufs=1) as cpool,
            tc.tile_pool(name="xin", bufs=XBUFS) as xpool,
            tc.tile_pool(name="tmp", bufs=TBUFS) as tpool,
            tc.tile_pool(name="psum", bufs=1, space="PSUM") as ppool,
            tc.tile_pool(name="psum2", bufs=2, space="PSUM") as ppool2,
        ):
            # ---- input DMAs. The HWDGE ring is FIFO, so order = stream
            # order: tiny state tensors first, then w chunks interleaved
            # with batch 0's x chunks (the first multiply needs only w
            # chunk 0 + x chunk 0).
            v_sb = cpool.tile([BPC, NN], f32)
            z_sb = cpool.tile([BPC, NN], f32)
            zo_sb = cpool.tile([BPC, NN], f32)
            nc.sync.dma_start(v_sb[:, :], v_h[:, :])
            nc.sync.dma_start(z_sb[:, :], z_h[:, :])
            nc.sync.dma_start(zo_sb[:, :], zo_h[:, :])

            wf_sb = cpool.tile([P, 4 * NN], f32)
            w_sb = cpool.tile([P, FD], f32)

            # per-batch indicator columns: ind[:, 8b + j] = (j == b)
            ind = cpool.tile([P, BPC * BPC], f32)
            nc.gpsimd.memset(ind[:, :], 0.0)
            for b in range(BPC):
                nc.gpsimd.memset(ind[:, 9 * b : 9 * b + 1], 1.0)

            ident = cpool.tile([BPC, BPC], f32)
            make_identity(nc, ident[:, :])

            # ---- lateral trace update ----
            zon_sb = cpool.tile([BPC, NN], f32)
            nc.vector.scalar_tensor_tensor(
                out=zon_sb[:, :], in0=zo_sb[:, :], scalar=BETA, in1=z_sb[:, :],
                op0=mybir.AluOpType.mult, op1=mybir.AluOpType.add,
            )

            # transpose z_out_new: 4x (8,128) -> (128,8)
            zonT = cpool.tile([P, 4 * BPC], f32)
            for t in range(4):
                psum_t = ppool2.tile([P, BPC], f32, tag="tr")
                nc.tensor.transpose(
                    psum_t[:, :], zon_sb[:, t * P : (t + 1) * P], ident[:, :]
                )
                nc.scalar.copy(zonT[:, t * BPC : (t + 1) * BPC], psum_t[:, :])

            # lateral drive: psum_lat[b,n] = sum_m zon[b,m] * Wf[m,n]
            nc.sync.dma_start(
                wf_sb[:, :].rearrange("p (t n) -> p t n", t=4), wf_r[:, :, :]
            )
            lat_tile = ppool.tile([BPC, NN], f32, tag="lat")
            for t in range(4):
                nc.tensor.matmul(
                    lat_tile[:, :],
                    zonT[:, t * BPC : (t + 1) * BPC],
                    wf_sb[:, t * NN : (t + 1) * NN],
                    start=(t == 0),
                    stop=(t == 3),
                )

            # ---- main loop: drive[b,n] = sum_k x[b,k,n]*w[k,n] ----
            # Per (b, chunk): DMA x chunk -> DVE product -> PE indicator-
            # matmul reduce into psum_drive row b. The first FOLD_CHUNKS
            # chunks per batch get a half-width DVE fold (4 slices -> 2),
            # trading cheap DVE adds for expensive fp32 PE columns.
            def fold_this(b, c):
                return c < 2

            total_mms = sum(
                (SPC // 2 if fold_this(b, c) else SPC)
                for b in range(BPC) for c in range(CHUNKS)
            )
            psum_drive = ppool.tile([BPC, NN], f32, tag="drive")
            mm_idx = 0
            for b in range(BPC):
                for c in range(CHUNKS):
                    cs = slice(c * CFD, (c + 1) * CFD)
                    if b == 0:
                        # stream w chunk c just ahead of the x chunk using it
                        nc.sync.dma_start(w_sb[:, cs], w_r[:, cs])
                    xc = xpool.tile([P, CFD], f32, tag="xc")
                    nc.sync.dma_start(xc[:, :], x_r[b, :, cs])
                    tm = tpool.tile([P, CFD], f32, tag="tm")
                    nc.vector.tensor_mul(tm[:, :], xc[:, :], w_sb[:, cs])
                    if fold_this(b, c):
                        # fold slices (s0,s1) += (s2,s3)
                        nc.vector.tensor_add(
                            tm[:, : CFD // 2], tm[:, : CFD // 2], tm[:, CFD // 2 :]
                        )
                    for j in range(SPC // 2 if fold_this(b, c) else SPC):
                        nc.tensor.matmul(
                            psum_drive[:, :],
                            ind[:, BPC * b : BPC * (b + 1)],
                            tm[:, j * NN : (j + 1) * NN],
                            start=(mm_idx == 0),
                            stop=(mm_idx == total_mms - 1),
                        )
                        mm_idx += 1

            # ---- epilogue ----
            t1 = cpool.tile([BPC, NN], f32)
            nc.vector.scalar_tensor_tensor(
                out=t1[:, :], in0=v_sb[:, :], scalar=ALPHA, in1=psum_drive[:, :],
                op0=mybir.AluOpType.mult, op1=mybir.AluOpType.add,
            )
            t2 = cpool.tile([BPC, NN], f32)
            nc.vector.scalar_tensor_tensor(
                out=t2[:, :], in0=z_sb[:, :], scalar=-V_TH, in1=lat_tile[:, :],
                op0=mybir.AluOpType.mult, op1=mybir.AluOpType.add,
            )
            vn_sb = cpool.tile([BPC, NN], f32)
            nc.vector.tensor_add(vn_sb[:, :], t1[:, :], t2[:, :])

            zn_sb = cpool.tile([BPC, NN], f32)
            nc.vector.tensor_scalar(
                out=zn_sb[:, :],
                in0=vn_sb[:, :],
                scalar1=V_TH,
                scalar2=None,
                op0=mybir.AluOpType.is_gt,
            )

            nc.sync.dma_start(out_h[0, :, :], vn_sb[:, :])
            nc.sync.dma_start(out_h[1, :, :], zn_sb[:, :])
            nc.sync.dma_start(out_h[2, :, :], zon_sb[:, :])

    return nc


def _make_wf(w: np.ndarray) -> np.ndarray:
    """Wf[m,n] = w[N_IN + m - (m>n), n] off-diagonal, 0 on the diagonal."""
    wl = w[N_IN:]
    m = np.arange(NN)[:, None]
    n = np.arange(NN)[None, :]
    idx = np.minimum(np.where(m > n, m - 1, m), NN - 2)
    return np.where(m == n, np.float32(0.0), wl[idx, n]).astype(np.float32)


def _make_in_maps(x, v, z, z_out, w):
    w_x = np.ascontiguousarray(w[:N_IN], dtype=np.float32)
    wf = _make_wf(np.asarray(w, dtype=np.float32))
    in_maps = []
    for c in range(NCORES):
        sl = slice(c * BPC, (c + 1) * BPC)
        in_maps.append(
            {
                "x": np.ascontiguousarray(x[sl], dtype=np.float32),
                "v": np.ascontiguousarray(v[sl], dtype=np.float32),
                "z": np.ascontiguousarray(z[sl], dtype=np.float32),
                "z_out": np.ascontiguousarray(z_out[sl], dtype=np.float32),
                "w": w_x,
                "wf": wf,
            }
        )
    return in_maps


def run(x, v, z, z_out, w, trace=False):
    """Build + run on the 8 NeuronCores; returns (output, BassKernelResults)."""
    from concourse.bass_utils import run_bass_kernel_spmd

    nc = _build_nc()
    if not nc.is_finalized():
        nc.finalize()
    in_maps = _make_in_maps(x, v, z, z_out, w)
    res = run_bass_kernel_spmd(nc, in_maps, core_ids=list(range(NCORES)), trace=trace)
    full = np.concatenate([r["out"] for r in res.results], axis=1)
    return np.ascontiguousarray(full, dtype=np.float32), res


def kernel(x, v, z, z_out, w):
    out, _ = run(x, v, z, z_out, w)
    return out
